# revision 22
# baseline (speedup 1.0000x reference)
"""Trainium2 Bass kernel for nn_DirectInjectionEncoder (moe_routing).

Strategy (8 NeuronCores):
  - The three big projection GEMMs (Wgate/Wup/Wdown, 10240->2560) are
    sharded over the output dim d_model=2560 -> 320 columns per core, so
    each core streams only 1/8 of the big weights from HBM. Every core
    computes its 320-column slice of all 16*36=576 rows per group.
  - Row L2-norms need the full 2560-dim row, so each core computes partial
    sums of squares for its slice; one tiny 8-core AllGather (15 cols x 128
    partitions) distributes the partials and every core reconstructs the
    full norm locally before scaling its slice.
  - The small projections (Wk/Wv, 640->2560) are data-parallel over the
    batch (weights replicated, norms core-local), and run together with the
    identity tokens inside the AllGather's latency window.
  - Identity tokens (9 of 14 slots/layer, first 2560 dims, no weights) are
    data-parallel over the batch: core c handles batches [2c, 2c+1] fully.
  - Host-side prep ("sharding") gathers token groups, pre-transposes the
    contraction dim onto partitions, and slices the weights per core.
  - GEMM operands stream as bf16 (KERNEL_GEMM_DT=f32r/f32 to override);
    PSUM accumulation and the norm math stay fp32. fp32 matmul runs at 1/4
    PE rate on TRN2, so bf16/f32r is 4x PE throughput; bf16 also halves
    the dominant DMA traffic.
"""

import os
import sys

sys.path.insert(0, "/opt/trn_rl_repo")

import numpy as np
import ml_dtypes

from concourse import bacc, bass, mybir
from concourse.bass_utils import run_bass_kernel_spmd
from concourse.tile import TileContext

D_MODEL = 2560
NUM_LAYERS = 36
TOKENS_PER_LAYER = 14
EPS = 1e-8
B = 16
N_CORES = 8
CORE_IDS = list(range(N_CORES))
D_SHARD = D_MODEL // N_CORES  # 320
ROWS = B * NUM_LAYERS  # 576
ROW_TILES = [(0, 128), (128, 128), (256, 128), (384, 128), (512, 64)]
ROWS_PC = ROWS // N_CORES  # 72 rows/core for the batch-parallel paths

IDENTITY_OFFSETS = np.array([0, 1, 2, 4, 6, 7, 8, 10, 13])
BIG_GROUPS = [(9, "Wgate"), (11, "Wup"), (12, "Wdown")]  # in_dim 10240, d-sharded
KV_GROUPS = [(3, "Wk"), (5, "Wv")]  # in_dim 640, batch-parallel
KV_IND = 640
BIG_IND = 10240
N_CHUNKS = D_MODEL // 512  # 5 output chunks for the kv path

ID_ROWS = (B // N_CORES) * NUM_LAYERS * len(IDENTITY_OFFSETS)  # 648
ID_TILES = [(0, 128), (128, 128), (256, 128), (384, 128), (512, 128), (640, 8)]
N_SSQ_COLS = len(BIG_GROUPS) * len(ROW_TILES)  # 15

F32 = mybir.dt.float32
AF = mybir.ActivationFunctionType

GEMM_MODE = os.environ.get("KERNEL_GEMM_DT", "bf16")
if GEMM_MODE == "bf16":
    GEMM_DT = mybir.dt.bfloat16
    GEMM_NP = ml_dtypes.bfloat16
    KB_BIG = 8  # k-tiles per DMA super-tile (~1.2 MB per xt transfer)
elif GEMM_MODE == "f32r":
    GEMM_DT = mybir.dt.float32r
    GEMM_NP = np.float32
    KB_BIG = 2
else:
    GEMM_DT = mybir.dt.float32
    GEMM_NP = np.float32
    KB_BIG = 2


def _positions(offset):
    return np.arange(NUM_LAYERS) * TOKENS_PER_LAYER + offset


def build_program():
    nc = bacc.Bacc("TRN2", num_devices=N_CORES)

    xt_d, wt_d, om_d = [], [], []
    for gi, (off, wname) in enumerate(BIG_GROUPS):
        xt_d.append(nc.declare_dram_parameter(f"xt_{gi}", [BIG_IND // (128 * KB_BIG), 128, KB_BIG * ROWS], GEMM_DT, isOutput=False))
        wt_d.append(nc.declare_dram_parameter(f"wt_{gi}", [BIG_IND // (128 * KB_BIG), 128, KB_BIG * D_SHARD], GEMM_DT, isOutput=False))
        om_d.append(nc.declare_dram_parameter(f"om_{gi}", [ROWS, D_SHARD], F32, isOutput=True))
    kvx_d, kvw_d, kvo_d = [], [], []
    for gi, (off, wname) in enumerate(KV_GROUPS):
        kvx_d.append(nc.declare_dram_parameter(f"kvx_{gi}", [128, 5 * ROWS_PC], GEMM_DT, isOutput=False))
        kvw_d.append(nc.declare_dram_parameter(f"kvw_{gi}", [128, 5 * D_MODEL], GEMM_DT, isOutput=False))
        kvo_d.append(nc.declare_dram_parameter(f"kvo_{gi}", [ROWS_PC, D_MODEL], F32, isOutput=True))
    idx_d = nc.declare_dram_parameter("id_x", [ID_ROWS, D_MODEL], F32, isOutput=False)
    ido_d = nc.declare_dram_parameter("out_id", [ID_ROWS, D_MODEL], F32, isOutput=True)

    with TileContext(nc) as tc:
        with (
            tc.tile_pool(name="xt", bufs=4) as xt_pool,
            tc.tile_pool(name="wt", bufs=4) as wt_pool,
            tc.tile_pool(name="sout", bufs=N_SSQ_COLS) as sout_pool,
            tc.tile_pool(name="scr", bufs=2) as scr_pool,
            tc.tile_pool(name="kvp", bufs=2) as kv_pool,
            tc.tile_pool(name="idp", bufs=3) as id_pool,
            tc.tile_pool(name="idscr", bufs=2) as idscr_pool,
            tc.tile_pool(name="small", bufs=1) as small_pool,
            tc.tile_pool(name="ps", bufs=8, space="PSUM") as psum_pool,
            tc.tile_pool(name="dram", bufs=1, space="DRAM") as dram_pool,
        ):
            ssq = small_pool.tile([128, N_SSQ_COLS], F32, tag="ssq")
            nc.vector.memset(ssq[:], 0.0)

            # Warmup AllGather: the first collective in a NEFF pays ~60us of
            # one-time setup. Fire a tiny dummy at kernel start so that cost
            # hides under the GEMM phase and the real AllGather at the tail
            # only pays the ~15us marginal cost.
            warm_sb = small_pool.tile([1, 16], F32, tag="warmsb")
            nc.vector.memset(warm_sb[:], 0.0)
            warm_in = dram_pool.tile([16], F32, tag="warmci")
            warm_out = dram_pool.tile([N_CORES, 16], F32, tag="warmco")
            nc.gpsimd.dma_start(out=warm_in[:], in_=warm_sb[0, :])
            nc.gpsimd.collective_compute(
                "AllGather",
                mybir.AluOpType.bypass,
                ins=[warm_in.opt()],
                outs=[warm_out.opt()],
                replica_groups=[CORE_IDS],
            )
            nc.gpsimd.dma_start(out=warm_sb[0, :], in_=warm_out[0, :])

            # Identity tile chain: emitted interleaved at group boundaries on
            # the sync ring, so the loads slot into the GEMM DMA stream and
            # the (DVE square / ACT scale) work runs while PE is busy.
            def identity_tile(t):
                t0, tw = ID_TILES[t]
                it = id_pool.tile([128, D_MODEL], F32, tag="idp", name=f"idp_{t}")
                iscr = idscr_pool.tile([128, D_MODEL], F32, tag="idscr", name=f"idscr_{t}")
                nc.scalar.dma_start(out=it[:tw, :], in_=idx_d[t0 : t0 + tw, :])
                issq = small_pool.tile([128, 1], F32, tag=f"idssq{t}", name=f"idssq_{t}")
                nc.scalar.activation(
                    iscr[:tw, :], it[:tw, :], AF.Square,
                    accum_out=issq[:tw, :],
                )
                inorm = small_pool.tile([128, 1], F32, tag=f"idnorm{t}", name=f"idnorm_{t}")
                nc.scalar.sqrt(inorm[:tw, :], issq[:tw, :])
                nc.scalar.activation(inorm[:tw, :], inorm[:tw, :], AF.Copy, bias=EPS)
                iscale = small_pool.tile([128, 1], F32, tag=f"idscale{t}", name=f"idscale_{t}")
                nc.vector.reciprocal(iscale[:tw, :], inorm[:tw, :])
                nc.scalar.activation(
                    it[:tw, :], it[:tw, :], AF.Copy,
                    scale=iscale[:tw, :],
                )
                nc.scalar.dma_start(out=ido_d[t0 : t0 + tw, :], in_=it[:tw, :])

            # k/v operand loads, emitted early at group boundaries so the kv
            # matmuls are not blocked on DMA at the tail.
            kv_tiles = {}

            def kv_load(gi):
                kvx = kv_pool.tile([128, 5, ROWS_PC], GEMM_DT, tag="kvx", name=f"kvx_{gi}")
                kvw = kv_pool.tile([128, 5, D_MODEL], GEMM_DT, tag="kvw", name=f"kvw_{gi}")
                nc.sync.dma_start(
                    out=kvx[:], in_=kvx_d[gi].rearrange("p (k c) -> p k c", k=5)
                )
                nc.sync.dma_start(
                    out=kvw[:], in_=kvw_d[gi].rearrange("p (k c) -> p k c", k=5)
                )
                kv_tiles[gi] = (kvx, kvw)

            # ---- big groups: d-sharded GEMMs, PSUM-accumulated over k ----
            souts = {}
            for gi, (off, wname) in enumerate(BIG_GROUPS):
                nk = BIG_IND // 128
                kb = KB_BIG
                nsup = nk // kb
                ps = [
                    psum_pool.tile([128, D_SHARD], F32, tag="ps", name=f"ps_{gi}_{ri}")
                    for ri in range(len(ROW_TILES))
                ]
                xt_view = xt_d[gi].rearrange("j p (kb c) -> j p kb c", kb=kb)
                wt_view = wt_d[gi].rearrange("j p (kb c) -> j p kb c", kb=kb)
                for j in range(nsup):
                    xt = xt_pool.tile([128, kb, ROWS], GEMM_DT, tag="xt", name=f"xt_{gi}_{j}")
                    wt = wt_pool.tile([128, kb, D_SHARD], GEMM_DT, tag="wt", name=f"wt_{gi}_{j}")
                    nc.sync.dma_start(out=xt[:], in_=xt_view[j])
                    nc.sync.dma_start(out=wt[:], in_=wt_view[j])
                    for k in range(kb):
                        kt = j * kb + k
                        for r, (r0, rw) in enumerate(ROW_TILES):
                            nc.tensor.matmul(
                                ps[r][:rw, :],
                                xt[:, k, r0 : r0 + rw],
                                wt[:, k, :],
                                start=(kt == 0),
                                stop=(kt == nk - 1),
                            )
                for r, (r0, rw) in enumerate(ROW_TILES):
                    col = gi * len(ROW_TILES) + r
                    so = sout_pool.tile([128, D_SHARD], F32, tag="sout", name=f"so_{gi}_{r}")
                    scr = scr_pool.tile([128, D_SHARD], F32, tag="scr", name=f"scr_{gi}_{r}")
                    nc.vector.tensor_copy(so[:rw, :], ps[r][:rw, :])
                    nc.scalar.activation(
                        scr[:rw, :], ps[r][:rw, :], AF.Square,
                        accum_out=ssq[:rw, col : col + 1],
                    )
                    souts[(gi, r)] = so
                # boundary work: slot identity/kv loads into the DMA stream
                if gi == 0:
                    kv_load(0)
                    identity_tile(0)
                    identity_tile(1)
                elif gi == 1:
                    kv_load(1)
                    identity_tile(2)
                    identity_tile(3)

            # ---- kick off the AllGather of big-group partial sums ----
            cc_in = dram_pool.tile([128, N_SSQ_COLS], F32, tag="ccin")
            cc_out = dram_pool.tile([N_CORES, 128, N_SSQ_COLS], F32, tag="ccout")
            nc.gpsimd.dma_start(out=cc_in[:], in_=ssq[:])
            nc.gpsimd.collective_compute(
                "AllGather",
                mybir.AluOpType.bypass,
                ins=[cc_in.opt()],
                outs=[cc_out.opt()],
                replica_groups=[CORE_IDS],
            )
            identity_tile(4)
            identity_tile(5)

            # ---- k/v: batch-parallel GEMMs with core-local norms; these and
            # the identity tokens fill the AllGather latency window ----
            for gi, (off, wname) in enumerate(KV_GROUPS):
                kvx, kvw = kv_tiles[gi]
                pcs = [
                    psum_pool.tile([128, 512], F32, tag="ps", name=f"pkv_{gi}_{ci}")
                    for ci in range(N_CHUNKS)
                ]
                for k in range(5):
                    for ci in range(N_CHUNKS):
                        nc.tensor.matmul(
                            pcs[ci][:ROWS_PC, :],
                            kvx[:, k, :],
                            kvw[:, k, ci * 512 : (ci + 1) * 512],
                            start=(k == 0),
                            stop=(k == 4),
                        )
                kvssq = small_pool.tile([128, N_CHUNKS], F32, tag=f"kvssq{gi}", name=f"kvssq_{gi}")
                for ci in range(N_CHUNKS):
                    kscr = scr_pool.tile([128, 512], F32, tag="scr", name=f"kscr_{gi}_{ci}")
                    nc.scalar.activation(
                        kscr[:ROWS_PC, :], pcs[ci][:ROWS_PC, :], AF.Square,
                        accum_out=kvssq[:ROWS_PC, ci : ci + 1],
                    )
                kvs = small_pool.tile([128, 1], F32, tag=f"kvs{gi}", name=f"kvs_{gi}")
                nc.vector.reduce_sum(kvs[:ROWS_PC, :], kvssq[:ROWS_PC, :], axis=mybir.AxisListType.X)
                nc.scalar.sqrt(kvs[:ROWS_PC, :], kvs[:ROWS_PC, :])
                nc.scalar.activation(kvs[:ROWS_PC, :], kvs[:ROWS_PC, :], AF.Copy, bias=EPS)
                kvsc = small_pool.tile([128, 1], F32, tag=f"kvsc{gi}", name=f"kvsc_{gi}")
                nc.vector.reciprocal(kvsc[:ROWS_PC, :], kvs[:ROWS_PC, :])
                kvo = kv_pool.tile([128, D_MODEL], F32, tag="kvo", name=f"kvo_{gi}")
                for ci in range(N_CHUNKS):
                    nc.scalar.activation(
                        kvo[:ROWS_PC, ci * 512 : (ci + 1) * 512],
                        pcs[ci][:ROWS_PC, :],
                        AF.Copy,
                        scale=kvsc[:ROWS_PC, :],
                    )
                nc.sync.dma_start(out=kvo_d[gi][:, :], in_=kvo[:ROWS_PC, :])

            # ---- AllGather readback, total norms, final scaling ----
            # Readback + output stores ride the ACT HWDGE ring so they don't
            # queue behind the kv/identity stores on the SP ring.
            ag = small_pool.tile([128, N_CORES * N_SSQ_COLS], F32, tag="ag")
            for rr in range(N_CORES):
                nc.scalar.dma_start(
                    out=ag[:, rr * N_SSQ_COLS : (rr + 1) * N_SSQ_COLS],
                    in_=cc_out[rr, :, :],
                )
            tsq = small_pool.tile([128, N_SSQ_COLS], F32, tag="tsq")
            nc.vector.tensor_add(tsq[:], ag[:, :N_SSQ_COLS], ag[:, N_SSQ_COLS : 2 * N_SSQ_COLS])
            for rr in range(2, N_CORES):
                nc.vector.tensor_add(
                    tsq[:], tsq[:], ag[:, rr * N_SSQ_COLS : (rr + 1) * N_SSQ_COLS]
                )
            norm = small_pool.tile([128, N_SSQ_COLS], F32, tag="norm")
            nc.scalar.sqrt(norm[:], tsq[:])
            nc.scalar.activation(norm[:], norm[:], AF.Copy, bias=EPS)
            scale = small_pool.tile([128, N_SSQ_COLS], F32, tag="scale")
            nc.vector.reciprocal(scale[:], norm[:])

            # Alternate DVE/ACT so both engines drain the scaling in parallel.
            for gi in range(len(BIG_GROUPS)):
                for r, (r0, rw) in enumerate(ROW_TILES):
                    col = gi * len(ROW_TILES) + r
                    so = souts[(gi, r)]
                    if col % 2 == 0:
                        nc.vector.tensor_scalar_mul(
                            so[:rw, :], so[:rw, :], scale[:rw, col : col + 1]
                        )
                    else:
                        nc.scalar.activation(
                            so[:rw, :], so[:rw, :], AF.Copy,
                            scale=scale[:rw, col : col + 1],
                        )
                    nc.scalar.dma_start(out=om_d[gi][r0 : r0 + rw, :], in_=so[:rw, :])

    nc.compile()
    return nc


_NC = None


def _get_nc():
    global _NC
    if _NC is None:
        _NC = build_program()
    return _NC


def _prep_inputs(lora_tokens, weights):
    """Host-side sharding: gather token groups, transpose contraction onto
    partitions, slice weights per core."""
    lora = np.ascontiguousarray(lora_tokens)

    def pack_supertiles(arr_t):
        # [K, C] -> [K/(128*kb), 128, kb*C]: dense per-partition runs so each
        # super-tile DMA is one fully-contiguous block.
        K, C = arr_t.shape
        kb = KB_BIG
        nsup = K // (128 * kb)
        return np.ascontiguousarray(
            arr_t.reshape(nsup, kb, 128, C).transpose(0, 2, 1, 3).reshape(nsup, 128, kb * C)
        )

    def pack_kv(arr_t):
        # [640, C] -> [128, 5*C]
        K, C = arr_t.shape
        return np.ascontiguousarray(
            arr_t.reshape(5, 128, C).transpose(1, 0, 2).reshape(128, 5 * C)
        )

    shared = {}
    for gi, (off, wname) in enumerate(BIG_GROUPS):
        pos = _positions(off)
        x = lora[:, pos, :].reshape(ROWS, BIG_IND)
        shared[f"xt_{gi}"] = pack_supertiles(x.T.astype(GEMM_NP))
    kv_x = {}
    for gi, (off, wname) in enumerate(KV_GROUPS):
        pos = _positions(off)
        kv_x[gi] = lora[:, pos, :KV_IND].reshape(ROWS, KV_IND)
        shared[f"kvw_{gi}"] = pack_kv(weights[wname].T.astype(GEMM_NP))

    id_pos = np.sort(np.concatenate([_positions(o) for o in IDENTITY_OFFSETS]))
    in_maps = []
    bpc = B // N_CORES
    for c in range(N_CORES):
        m = dict(shared)
        for gi, (off, wname) in enumerate(BIG_GROUPS):
            wsl = weights[wname][c * D_SHARD : (c + 1) * D_SHARD, :]  # [320, 10240]
            m[f"wt_{gi}"] = pack_supertiles(wsl.T.astype(GEMM_NP))
        for gi in range(len(KV_GROUPS)):
            m[f"kvx_{gi}"] = pack_kv(
                kv_x[gi][c * ROWS_PC : (c + 1) * ROWS_PC, :].T.astype(GEMM_NP)
            )
        m["id_x"] = np.ascontiguousarray(
            lora[c * bpc : (c + 1) * bpc, :, :][:, id_pos, :D_MODEL]
        ).reshape(ID_ROWS, D_MODEL)
        in_maps.append(m)
    return in_maps, id_pos


def run(inputs, trace=False):
    nc = _get_nc()
    weights = {k: inputs[k] for k in ("Wk", "Wv", "Wgate", "Wup", "Wdown")}
    in_maps, id_pos = _prep_inputs(inputs["lora_tokens"], weights)
    res = run_bass_kernel_spmd(nc, in_maps, CORE_IDS, trace=trace)

    out = np.zeros((B, NUM_LAYERS * TOKENS_PER_LAYER, D_MODEL), dtype=np.float32)
    bpc = B // N_CORES
    for c in range(N_CORES):
        r = res.results[c]
        out[c * bpc : (c + 1) * bpc, id_pos, :] = r["out_id"].reshape(
            bpc, len(id_pos), D_MODEL
        )
        for gi, (off, wname) in enumerate(BIG_GROUPS):
            pos = _positions(off)
            out[:, pos, c * D_SHARD : (c + 1) * D_SHARD] = r[f"om_{gi}"].reshape(
                B, NUM_LAYERS, D_SHARD
            )
        for gi, (off, wname) in enumerate(KV_GROUPS):
            pos = _positions(off)
            out[c * bpc : (c + 1) * bpc, pos, :] = r[f"kvo_{gi}"].reshape(
                bpc, NUM_LAYERS, D_MODEL
            )
    return out, res


def kernel(**inputs) -> np.ndarray:
    out, _ = run(inputs, trace=False)
    return out
